# revision 1
# baseline (speedup 1.0000x reference)
"""Multi-head causal attention (B=2, T=2048, C=1024, H=16, HD=64) on 8 TRN2 cores.

Sharding: core i -> batch b = i // 4, head-group g = i % 4 (heads 4g..4g+3).
Each core computes q/k/v projections for its 4 heads, causal softmax
attention, and a PARTIAL output projection against its slice of Wp.
Host sums the 4 partial projections per batch and adds the bias.

Device layout (per core):
  xT   [C, T]      x[b] transposed on host; partition dim = channel chunk.
  qT/kT stored per head-PAIR as [128, T] (two heads stacked on partitions).
  scores computed transposed: St[s, t] = k @ qT  (lhsT = kT, rhs = qT).
  P = exp(St * scale); causal handled by block skipping + a [128,128]
  lower-tri mask on diagonal blocks + memset of fully-masked columns.
  AV matmul: lhsT = V_ext [s, 65] (V plus a ones column), rhs = P[s, t]
  -> outT_ext [65, t] accumulated over s-chunks in PSUM; row 64 is the
  softmax denominator. Normalize with reciprocal + partition_broadcast.
  Projection: lhsT = outT pair chunks [128, t-tile], rhs = Wp rows.

All matmuls use float32r (TF32 on the PE) for 4x throughput over fp32.
"""

import numpy as np
import ml_dtypes
from contextlib import ExitStack

import concourse.bass as bass
from concourse import bacc
import concourse.mybir as mybir
import concourse.tile as tile
from concourse.bass_utils import run_bass_kernel_spmd

B, T, C, H, HD = 2, 2048, 1024, 16, 64
NCORES = 8
NH = 4               # heads per core
NPAIR = 2            # head pairs per core
NCH = C // 128       # 8 contraction chunks of 128
TBW = 512            # t-block width for scores/AV
NTB = T // TBW       # 4
NST = T // 128       # 16 s-tiles
NTT = T // 128       # 16 t-tiles for projection
SCALE = float(HD) ** -0.5

f32 = mybir.dt.float32
f32r = mybir.dt.float32r
bf16 = mybir.dt.float16
AF = mybir.ActivationFunctionType

# exec results of the last run (exec_time_ns etc.), for test harnesses
LAST_RESULTS = None


def build_program() -> bass.Bass:
    nc = bacc.Bacc("TRN2", target_bir_lowering=False, debug=False)

    xT_d = nc.dram_tensor("xT", [C, T], bf16, kind="ExternalInput")
    wqk_d = nc.dram_tensor("wqk", [128, NPAIR * 2 * NCH * 128], bf16,
                           kind="ExternalInput")
    wv_d = nc.dram_tensor("wv", [128, NCH * NH * HD], bf16, kind="ExternalInput")
    wp_d = nc.dram_tensor("wp", [128, NPAIR * C], bf16, kind="ExternalInput")
    tri_d = nc.dram_tensor("tri", [128, 640], bf16, kind="ExternalInput")
    ones_d = nc.dram_tensor("ones", [128, NH], bf16, kind="ExternalInput")
    out_d = nc.dram_tensor("out", [T, C], f32, kind="ExternalOutput")

    with tile.TileContext(nc) as tc:
        with ExitStack() as ctx:
            persist = ctx.enter_context(tc.tile_pool(name="persist", bufs=1))
            pt_pool = ctx.enter_context(tc.tile_pool(name="pt", bufs=6))
            rec_pool = ctx.enter_context(tc.tile_pool(name="rec", bufs=2))
            bc_pool = ctx.enter_context(tc.tile_pool(name="bc", bufs=2))
            pjs_pool = ctx.enter_context(tc.tile_pool(name="pjs", bufs=3))
            # "mm" slots are [128,1024] (2 banks each); qkv tiles borrow them
            ps_mm = ctx.enter_context(
                tc.tile_pool(name="ps_mm", bufs=2, space="PSUM"))
            ps_opj = ctx.enter_context(
                tc.tile_pool(name="ps_opj", bufs=4, space="PSUM"))

            # ---- persistent SBUF tensors ----
            wqk_sb = persist.tile([128, NPAIR * 2 * NCH * 128], bf16, tag="wqk")
            nc.sync.dma_start(wqk_sb[:], wqk_d[:])
            wv_sb = persist.tile([128, NCH * NH * HD], bf16, tag="wv")
            nc.sync.dma_start(wv_sb[:], wv_d[:])
            wp_sb = persist.tile([128, NPAIR * C], bf16, tag="wp")
            nc.sync.dma_start(wp_sb[:], wp_d[:])
            tri_sb = persist.tile([128, 640], bf16, tag="tri")
            nc.sync.dma_start(tri_sb[:], tri_d[:])
            ones_sb = persist.tile([128, NH], bf16, tag="ones")
            nc.sync.dma_start(ones_sb[:], ones_d[:])

            xT_sb = []
            for ch in range(NCH):
                t_ = persist.tile([128, T], bf16, tag=f"xT{ch}", name=f"xT{ch}")
                nc.sync.dma_start(t_[:], xT_d[ch * 128:(ch + 1) * 128, :])
                xT_sb.append(t_)

            # qkT[p][0] = qT pair tile [128, T]; [p][1] = kT
            qkT = [[persist.tile([128, T], bf16, tag=f"qk{p}{t_i}", name=f"qk{p}{t_i}")
                    for t_i in range(2)] for p in range(NPAIR)]
            # v tiles: [128, NH*65]; per head h: cols [h*65, h*65+64) = V,
            # col h*65+64 = 1.0 (ones column for the denominator)
            v_sb = [persist.tile([128, NH * 65], bf16, tag=f"v{st}", name=f"v{st}")
                    for st in range(NST)]
            # attention output transposed, per pair [128, T]
            oT = [persist.tile([128, T], bf16, tag=f"o{p}", name=f"o{p}") for p in range(NPAIR)]

            wqk3 = wqk_sb[:].rearrange("p (a b c m) -> p a b c m",
                                       a=NPAIR, b=2, c=NCH)
            wv3 = wv_sb[:].rearrange("p (c n) -> p c n", c=NCH)
            wp3 = wp_sb[:].rearrange("p (a n) -> p a n", a=NPAIR)

            # ---- fused per-t-block pipeline: qkv(tb) then attention(tb);
            # qkv(tb+1) matmuls fill attention(tb) dependency stalls ----
            for tb in range(NTB):
                nst = 4 * (tb + 1)
                tsl = slice(tb * TBW, (tb + 1) * TBW)

                # q/k projections for this t-block (both pairs)
                for p in range(NPAIR):
                    for t_i in range(2):
                        ps = ps_mm.tile([128, 2 * TBW], f32, tag="mm",
                                        name="ps")
                        for ch in range(NCH):
                            nc.tensor.matmul(
                                ps[:, 0:TBW],
                                wqk3[:, p, t_i, ch, :],
                                xT_sb[ch][:, tb * TBW:(tb + 1) * TBW],
                                start=(ch == 0), stop=(ch == NCH - 1))
                        nc.vector.tensor_copy(
                            qkT[p][t_i][:, tb * TBW:(tb + 1) * TBW],
                            ps[:, 0:TBW])

                # V for the 4 s-tiles of this t-block
                for st in range(4 * tb, 4 * tb + 4):
                    ps = ps_mm.tile([128, 2 * TBW], f32, tag="mm", name="ps")
                    for ch in range(NCH):
                        nc.tensor.matmul(
                            ps[:, 0:NH * HD],
                            xT_sb[ch][:, st * 128:(st + 1) * 128],
                            wv3[:, ch, :],
                            start=(ch == 0), stop=(ch == NCH - 1))
                    vt = v_sb[st]
                    vt3 = vt[:].rearrange("p (h e) -> p h e", e=65)
                    nc.vector.tensor_copy(
                        vt3[:, :, 64:65],
                        ones_sb[:].rearrange("p (h e) -> p h e", e=1))
                    nc.vector.tensor_copy(
                        vt3[:, :, 0:64],
                        ps[:, 0:NH * HD].rearrange("p (h e) -> p h e", e=64))

                for p in range(NPAIR):
                    qT, kT = qkT[p][0], qkT[p][1]
                    rps = [slice(0, 64), slice(64, 128)]
                    opss = [ps_opj.tile([128, TBW], f32, tag="opj",
                                      name=f"ops{tb}{p}{s}") for s in range(2)]
                    for st in range(nst):
                        scp = ps_mm.tile([128, 2 * TBW], f32, tag="mm",
                                         name="scp")
                        for sub in range(2):
                            nc.tensor.matmul(
                                scp[:, sub * TBW:(sub + 1) * TBW],
                                kT[rps[sub], st * 128:(st + 1) * 128],
                                qT[rps[sub], tsl],
                                start=True, stop=True,
                                tile_position=(sub * 64, 0))
                        pt = pt_pool.tile([128, 2 * TBW], bf16, tag="pt")
                        nc.scalar.activation(pt[:], scp[:], AF.Exp, scale=SCALE)
                        jrel = st - 4 * tb
                        if jrel >= 0:  # diagonal s-tile
                            # zero region + causal triangle in one multiply:
                            # tri strip [128, 640]: tri[s, v] = (v >= s + 512)
                            w = (jrel + 1) * 128
                            for sub in range(2):
                                nc.vector.tensor_mul(
                                    pt[:, sub * TBW:sub * TBW + w],
                                    pt[:, sub * TBW:sub * TBW + w],
                                    tri_sb[:, 512 - jrel * 128:640])
                        vt3 = v_sb[st][:].rearrange("p (h e) -> p h e", e=65)
                        lo = jrel * 128 if jrel > 0 else 0
                        for sub in range(2):
                            nc.tensor.matmul(
                                opss[sub][0:65, lo:TBW],
                                vt3[:, 2 * p + sub, :],
                                pt[:, sub * TBW + lo:(sub + 1) * TBW],
                                start=(st == 0), stop=(st == nst - 1))
                    # normalize: oT[d, t] = ops[d, t] * (1 / ops[64, t])
                    for sub in range(2):
                        ops = opss[sub]
                        rp = rps[sub]
                        rs = rec_pool.tile([1, TBW], f32, tag="rs")
                        nc.vector.tensor_copy(rs[:], ops[64:65, :])
                        rec = rec_pool.tile([1, TBW], f32, tag="rec")
                        nc.vector.reciprocal_approx_fast(rec[:], rs[:])
                        bc = bc_pool.tile([64, TBW], f32, tag="bc")
                        nc.gpsimd.partition_broadcast(bc[:], rec[:])
                        nc.vector.tensor_mul(oT[p][rp, tsl], ops[0:64, :],
                                             bc[:])



            # ---- projection, emitted last: fills PE gaps in the
            # ACT-bound attention stretches; deps allow early execution ----
            for tt in range(NTT):
                for cb in range(2):
                    pj = ps_opj.tile([128, 512], f32, tag="opj", name="pj")
                    for p in range(NPAIR):
                        nc.tensor.matmul(
                            pj[:],
                            oT[p][:, tt * 128:(tt + 1) * 128],
                            wp3[:, p, cb * 512:(cb + 1) * 512],
                            start=(p == 0), stop=(p == NPAIR - 1))
                    pjs = pjs_pool.tile([128, 512], f32, tag="pjs")
                    nc.vector.tensor_copy(pjs[:], pj[:])
                    nc.sync.dma_start(
                        out_d[tt * 128:(tt + 1) * 128,
                              cb * 512:(cb + 1) * 512],
                        pjs[:])

    nc.compile()
    return nc


def _pack_core_inputs(core, x, Wq, Wk, Wv, Wp):
    b, g = core // 4, core % 4
    hs = [4 * g + i for i in range(NH)]

    xT = np.ascontiguousarray(x[b].T)

    wqk = np.empty((128, NPAIR, 2, NCH, 128), np.float32)
    for p in range(NPAIR):
        for t_i, W in enumerate((Wq, Wk)):
            pair = np.concatenate([W[hs[2 * p]], W[hs[2 * p + 1]]], axis=1)
            for ch in range(NCH):
                wqk[:, p, t_i, ch, :] = pair[ch * 128:(ch + 1) * 128, :]

    wv4 = np.concatenate([Wv[h] for h in hs], axis=1)  # [C, 256]
    wv = np.empty((128, NCH, NH * HD), np.float32)
    for ch in range(NCH):
        wv[:, ch, :] = wv4[ch * 128:(ch + 1) * 128]

    wp = np.empty((128, NPAIR, C), np.float32)
    for p in range(NPAIR):
        rows = np.r_[hs[2 * p] * HD:(hs[2 * p] + 1) * HD,
                     hs[2 * p + 1] * HD:(hs[2 * p + 1] + 1) * HD]
        wp[:, p, :] = Wp[rows, :]

    v_idx = np.arange(640)[None, :]
    s_idx = np.arange(128)[:, None]
    tri = (v_idx >= s_idx + 512)

    bf = np.float16
    return {
        "xT": xT.astype(bf),
        "wqk": np.ascontiguousarray(wqk.reshape(128, -1)).astype(bf),
        "wv": np.ascontiguousarray(wv.reshape(128, -1)).astype(bf),
        "wp": np.ascontiguousarray(wp.reshape(128, -1)).astype(bf),
        "tri": tri.astype(bf),
        "ones": np.ones((128, NH), bf),
    }


def kernel(x, Wq, Wk, Wv, Wp, bp, _trace=False):
    global LAST_RESULTS
    x = np.asarray(x, np.float32)
    Wq = np.asarray(Wq, np.float32)
    Wk = np.asarray(Wk, np.float32)
    Wv = np.asarray(Wv, np.float32)
    Wp = np.asarray(Wp, np.float32)
    bp = np.asarray(bp, np.float32)

    nc = build_program()
    in_maps = [_pack_core_inputs(c, x, Wq, Wk, Wv, Wp) for c in range(NCORES)]
    kres = run_bass_kernel_spmd(nc, in_maps, list(range(NCORES)),
                                trace=_trace)
    LAST_RESULTS = kres
    res = kres.results

    out = np.empty((B, T, C), np.float32)
    for b in range(B):
        acc = np.zeros((T, C), np.float64)
        for g in range(4):
            acc += res[4 * b + g]["out"]
        out[b] = (acc + bp.astype(np.float64)).astype(np.float32)
    return out



# revision 5
# speedup vs baseline: 1.3762x; 1.3762x over previous
"""Multi-head causal attention (B=2, T=2048, C=1024, H=16, HD=64) on 8 TRN2 cores.

Sharding: core i -> batch b = i // 4, head-group g = i % 4 (heads 4g..4g+3).
Each core computes q/k/v projections for its 4 heads, causal softmax
attention, and a PARTIAL output projection against its slice of Wp.
Host sums the 4 partial projections per batch and adds the bias.

Device layout (per core):
  xT   [C, T]      x[b] transposed on host; packed (tb, ch)-major so the
                   first t-block's slice can be DMA'd (and computed) first.
  qT/kT stored per head-PAIR as [128, T] (two heads stacked on partitions).
  scores computed transposed: St[s, t] = k @ qT  (lhsT = kT, rhs = qT).
  P = exp(St * scale); causal via block skipping + windowed exp + a
  [128,128] lower-tri mask multiply on the diagonal 128-col strip.
  AV matmul: lhsT = V_ext [s, 65] (V plus a ones column), rhs = P[s, t]
  -> outT_ext [65, t] accumulated over s-chunks in PSUM; row 64 is the
  softmax denominator. Normalize with reciprocal + partition_broadcast.
  Projection: lhsT = outT pair chunks [128, t-tile], rhs = Wp rows.

Scheduling: the attention stream is ACT(exp)-bound; scores/exp run with a
2-iteration lead over the AV matmuls so the ACT pipeline never waits on
the PE FIFO, and qkv-projection / output-projection matmuls are emitted
as paced filler between attention iterations to soak up PE slack.
A few warm-up matmuls run during the initial DMA so HAM un-throttles
the PE clock before real work starts.
"""

import numpy as np
from collections import deque
from contextlib import ExitStack

import concourse.bass as bass
from concourse import bacc
import concourse.mybir as mybir
import concourse.tile as tile
from concourse.bass_utils import run_bass_kernel_spmd

B, T, C, H, HD = 2, 2048, 1024, 16, 64
NCORES = 8
NH = 4               # heads per core
NPAIR = 2            # head pairs per core
NCH = C // 128       # 8 contraction chunks of 128
TBW = 512            # t-block width for scores/AV
NTB = T // TBW       # 4
NST = T // 128       # 16 s-tiles
SCALE = float(HD) ** -0.5

f32 = mybir.dt.float32
f16 = mybir.dt.float16
AF = mybir.ActivationFunctionType

# exec results of the last run (exec_time_ns etc.), for test harnesses
LAST_RESULTS = None


def build_program() -> bass.Bass:
    nc = bacc.Bacc("TRN2", target_bir_lowering=False, debug=False)

    # xT packed (tb, ch)-major: block (tb, ch) = x[b].T[ch*128:, tb*TBW:]
    xT_d = nc.dram_tensor("xT", [128, NTB * NCH * TBW], f16,
                          kind="ExternalInput")
    wqk_d = nc.dram_tensor("wqk", [128, NPAIR * 2 * NCH * 128], f16,
                           kind="ExternalInput")
    wv_d = nc.dram_tensor("wv", [128, NCH * NH * HD], f16, kind="ExternalInput")
    wp_d = nc.dram_tensor("wp", [128, NPAIR * C], f16, kind="ExternalInput")
    tri_d = nc.dram_tensor("tri", [128, 128], f16, kind="ExternalInput")
    out_d = nc.dram_tensor("out", [T, C], f32, kind="ExternalOutput")

    with tile.TileContext(nc) as tc:
        with ExitStack() as ctx:
            persist = ctx.enter_context(tc.tile_pool(name="persist", bufs=1))
            pt_pool = ctx.enter_context(tc.tile_pool(name="pt", bufs=4))
            rec_pool = ctx.enter_context(tc.tile_pool(name="rec", bufs=2))
            bc_pool = ctx.enter_context(tc.tile_pool(name="bc", bufs=2))
            pjs_pool = ctx.enter_context(tc.tile_pool(name="pjs", bufs=3))
            # PSUM budget (8 banks): ps512 2 + ps_sc 4 + ps_av 2
            ps512 = ctx.enter_context(
                tc.tile_pool(name="ps512", bufs=2, space="PSUM"))
            ps_sc = ctx.enter_context(
                tc.tile_pool(name="ps_sc", bufs=2, space="PSUM"))
            ps_av = ctx.enter_context(
                tc.tile_pool(name="ps_av", bufs=2, space="PSUM"))

            # ---- persistent SBUF tensors; DMA order = criticality ----
            tri_sb = persist.tile([128, 128], f16, tag="tri")
            nc.sync.dma_start(tri_sb[:], tri_d[:])

            xT_sb = []            # per tb: [128, NCH*TBW]
            for tb in range(NTB):
                t_ = persist.tile([128, NCH * TBW], f16, tag=f"xT{tb}",
                                  name=f"xT{tb}")
                xT_sb.append(t_)
            # tb0 split per-chunk so the first q/k matmuls start early
            for ch in range(NCH):
                nc.sync.dma_start(
                    xT_sb[0][:, ch * TBW:(ch + 1) * TBW],
                    xT_d[:, ch * TBW:(ch + 1) * TBW])
            wqk_sb = persist.tile([128, NPAIR * 2 * NCH * 128], f16, tag="wqk")
            nc.sync.dma_start(wqk_sb[:], wqk_d[:])
            wv_sb = persist.tile([128, NCH * NH * HD], f16, tag="wv")
            nc.sync.dma_start(wv_sb[:], wv_d[:])
            wp_sb = persist.tile([128, NPAIR * C], f16, tag="wp")
            nc.sync.dma_start(wp_sb[:], wp_d[:])
            for tb in range(1, NTB):
                nc.sync.dma_start(
                    xT_sb[tb][:],
                    xT_d[:, tb * NCH * TBW:(tb + 1) * NCH * TBW])

            # qkT[p][0] = qT pair tile [128, T]; [p][1] = kT
            qkT = [[persist.tile([128, T], f16, tag=f"qk{p}{t_i}",
                                 name=f"qk{p}{t_i}")
                    for t_i in range(2)] for p in range(NPAIR)]
            # v tiles: [128, NH*65]; per head h: cols [h*65, h*65+64) = V,
            # col h*65+64 = 1.0 (ones column for the denominator)
            v_sb = [persist.tile([128, NH * 65], f16, tag=f"v{st}",
                                 name=f"v{st}")
                    for st in range(NST)]
            # ones columns never change: set once at startup
            for st in range(NST):
                vt3 = v_sb[st][:].rearrange("p (h e) -> p h e", e=65)
                nc.vector.memset(vt3[:, :, 64:65], 1.0)
            # attention output transposed, per pair [128, T]
            oT = [persist.tile([128, T], f16, tag=f"o{p}", name=f"o{p}")
                  for p in range(NPAIR)]

            wqk3 = wqk_sb[:].rearrange("p (a b c m) -> p a b c m",
                                       a=NPAIR, b=2, c=NCH)
            wv3 = wv_sb[:].rearrange("p (c n) -> p c n", c=NCH)
            wp3 = wp_sb[:].rearrange("p (a n) -> p a n", a=NPAIR)
            xT3 = [x_[:].rearrange("p (c w) -> p c w", c=NCH) for x_ in xT_sb]

            # ---- PE warm-up: HAM un-throttles after ~3.4us of activity ----
            for _ in range(24):
                ps = ps512.tile([128, TBW], f32, tag="mm", name="warm")
                nc.tensor.matmul(ps[:, 0:128], tri_sb[:], tri_sb[:],
                                 start=True, stop=True)

            # ---- filler machinery: qkv / projection work paced into the
            # ACT-bound attention stream ----
            state = {"pe": 0.0, "act": 0.0}
            filler = deque()      # (tag_tb, pe_cost_ns, emit_fn)
            qk_done = {}          # (tb, p) -> bool

            def emit_qk(tb, p, t_i):
                def fn():
                    ps = ps512.tile([128, TBW], f32, tag="mm", name="ps")
                    for ch in range(NCH):
                        nc.tensor.matmul(
                            ps[:], wqk3[:, p, t_i, ch, :],
                            xT3[tb][:, ch, :],
                            start=(ch == 0), stop=(ch == NCH - 1))
                    nc.vector.tensor_copy(
                        qkT[p][t_i][:, tb * TBW:(tb + 1) * TBW], ps[:])
                    if t_i == 1:
                        qk_done[(tb, p)] = True
                return fn

            def emit_v(st):
                def fn():
                    tb = st // 4
                    j = st % 4
                    ps = ps512.tile([128, TBW], f32, tag="mm", name="ps")
                    for ch in range(NCH):
                        nc.tensor.matmul(
                            ps[:, 0:NH * HD],
                            xT3[tb][:, ch, j * 128:(j + 1) * 128],
                            wv3[:, ch, :],
                            start=(ch == 0), stop=(ch == NCH - 1))
                    vt3 = v_sb[st][:].rearrange("p (h e) -> p h e", e=65)
                    nc.vector.tensor_copy(
                        vt3[:, :, 0:64],
                        ps[:, 0:NH * HD].rearrange("p (h e) -> p h e", e=64))
                return fn

            def emit_proj(tt):
                def fn():
                    pjs = pjs_pool.tile([128, 2 * TBW], f32, tag="pjs")
                    for cb in range(2):
                        pj = ps512.tile([128, TBW], f32, tag="mm", name="pj")
                        for p in range(NPAIR):
                            nc.tensor.matmul(
                                pj[:],
                                oT[p][:, tt * 128:(tt + 1) * 128],
                                wp3[:, p, cb * TBW:(cb + 1) * TBW],
                                start=(p == 0), stop=(p == NPAIR - 1))
                        nc.vector.tensor_copy(
                            pjs[:, cb * TBW:(cb + 1) * TBW], pj[:])
                    nc.sync.dma_start(out_d[tt * 128:(tt + 1) * 128, :],
                                      pjs[:])
                return fn

            def queue_qkv(tb):
                for t_i in range(2):
                    filler.append((tb, 1730, emit_qk(tb, 0, t_i)))
                for st in range(4 * tb, 4 * tb + 4):
                    filler.append((tb, 1050, emit_v(st)))
                for t_i in range(2):
                    filler.append((tb, 1730, emit_qk(tb, 1, t_i)))
                qk_done[(tb, 0)] = False
                qk_done[(tb, 1)] = False

            def pop1():
                tag, cost, fn = filler.popleft()
                fn()
                state["pe"] += cost

            def pop_budget():
                while filler and \
                        state["pe"] + filler[0][1] < state["act"] - 200:
                    pop1()

            def pop_forced(tb):
                while filler and filler[0][0] <= tb:
                    pop1()

            # ---- main t-block loop ----
            rps = [slice(0, 64), slice(64, 128)]
            queue_qkv(0)
            for tb in range(NTB):
                nst = 4 * (tb + 1)
                tsl = slice(tb * TBW, (tb + 1) * TBW)
                if tb + 1 < NTB:
                    queue_qkv(tb + 1)

                for p in range(NPAIR):
                    while not qk_done[(tb, p)]:
                        pop1()
                    qT, kT = qkT[p][0], qkT[p][1]
                    opss = [ps_av.tile([128, TBW], f32, tag="av",
                                       name=f"av{tb}{p}{s}")
                            for s in range(2)]
                    pts = {}

                    def sc_exp(st):
                        jrel = st - 4 * tb
                        lo = max(jrel, 0) * 128
                        scp = ps_sc.tile([128, 2 * TBW], f32, tag="sc",
                                         name="scp")
                        for sub in range(2):
                            nc.tensor.matmul(
                                scp[:, sub * TBW + lo:(sub + 1) * TBW],
                                kT[rps[sub], st * 128:(st + 1) * 128],
                                qT[rps[sub], tb * TBW + lo:(tb + 1) * TBW],
                                start=True, stop=True,
                                tile_position=(sub * 64, 0))
                        pt = pt_pool.tile([128, 2 * TBW], f16, tag="pt")
                        # 2-row strided AP skips the dead cols between subs
                        pt3 = pt[:].rearrange("p (s w) -> p s w", s=2)
                        scp3 = scp[:].rearrange("p (s w) -> p s w", s=2)
                        nc.scalar.activation(pt3[:, :, lo:TBW],
                                             scp3[:, :, lo:TBW],
                                             AF.Exp, scale=SCALE)
                        pts[st] = (pt, lo)
                        state["pe"] += 2 * (TBW - lo) / 2.4 + 40
                        state["act"] += (2 * (TBW - lo) + 579) / 1.2

                    def av(st):
                        pt, lo = pts.pop(st)
                        jrel = st - 4 * tb
                        if jrel >= 0:
                            # lower-tri mask on the 128-col diagonal strip
                            for sub in range(2):
                                nc.vector.tensor_mul(
                                    pt[:, sub * TBW + lo:sub * TBW + lo + 128],
                                    pt[:, sub * TBW + lo:sub * TBW + lo + 128],
                                    tri_sb[:])
                        vt3 = v_sb[st][:].rearrange("p (h e) -> p h e", e=65)
                        for sub in range(2):
                            nc.tensor.matmul(
                                opss[sub][0:65, lo:TBW],
                                vt3[:, 2 * p + sub, :],
                                pt[:, sub * TBW + lo:(sub + 1) * TBW],
                                start=(st == 0), stop=(st == nst - 1))
                        state["pe"] += 2 * (TBW - lo) / 2.4 + 90

                    LEAD = 2
                    for st in range(nst):
                        sc_exp(st)
                        if filler and filler[0][0] <= tb:
                            pop1()
                        if st >= LEAD:
                            av(st - LEAD)
                            pop_budget()
                    for st in range(max(nst - LEAD, 0), nst):
                        av(st)

                    # normalize: oT[d, t] = av[d, t] * (1 / av[64, t])
                    for sub in range(2):
                        ops = opss[sub]
                        rs = rec_pool.tile([1, TBW], f32, tag="rs")
                        nc.vector.tensor_copy(rs[:], ops[64:65, :])
                        rec = rec_pool.tile([1, TBW], f32, tag="rec")
                        nc.vector.reciprocal_approx_fast(rec[:], rs[:])
                        bcb = bc_pool.tile([64, TBW], f32, tag="bc")
                        nc.gpsimd.partition_broadcast(bcb[:], rec[:])
                        nc.vector.tensor_mul(oT[p][rps[sub], tsl],
                                             ops[0:64, :], bcb[:])

                # output projection for this t-block's 4 t-tiles (as filler)
                for tt in range(4 * tb, 4 * tb + 4):
                    filler.append((tb + 1, 900, emit_proj(tt)))

            while filler:
                pop1()

    nc.compile()
    return nc


def _pack_core_inputs(core, x, Wq, Wk, Wv, Wp):
    b, g = core // 4, core % 4
    hs = [4 * g + i for i in range(NH)]

    xT = np.ascontiguousarray(x[b].T)  # [C, T]
    xTp = np.empty((128, NTB, NCH, TBW), np.float32)
    for tb in range(NTB):
        for ch in range(NCH):
            xTp[:, tb, ch, :] = xT[ch * 128:(ch + 1) * 128,
                                   tb * TBW:(tb + 1) * TBW]

    wqk = np.empty((128, NPAIR, 2, NCH, 128), np.float32)
    for p in range(NPAIR):
        for t_i, W in enumerate((Wq, Wk)):
            pair = np.concatenate([W[hs[2 * p]], W[hs[2 * p + 1]]], axis=1)
            for ch in range(NCH):
                wqk[:, p, t_i, ch, :] = pair[ch * 128:(ch + 1) * 128, :]

    wv4 = np.concatenate([Wv[h] for h in hs], axis=1)  # [C, 256]
    wv = np.empty((128, NCH, NH * HD), np.float32)
    for ch in range(NCH):
        wv[:, ch, :] = wv4[ch * 128:(ch + 1) * 128]

    wp = np.empty((128, NPAIR, C), np.float32)
    for p in range(NPAIR):
        rows = np.r_[hs[2 * p] * HD:(hs[2 * p] + 1) * HD,
                     hs[2 * p + 1] * HD:(hs[2 * p + 1] + 1) * HD]
        wp[:, p, :] = Wp[rows, :]

    c_idx = np.arange(128)[None, :]
    s_idx = np.arange(128)[:, None]
    tri = (c_idx >= s_idx)  # [128, 128] lower-tri inclusive, t >= s

    bf = np.float16
    return {
        "xT": np.ascontiguousarray(xTp.reshape(128, -1)).astype(bf),
        "wqk": np.ascontiguousarray(wqk.reshape(128, -1)).astype(bf),
        "wv": np.ascontiguousarray(wv.reshape(128, -1)).astype(bf),
        "wp": np.ascontiguousarray(wp.reshape(128, -1)).astype(bf),
        "tri": tri.astype(bf),
    }


def kernel(x, Wq, Wk, Wv, Wp, bp, _trace=False):
    global LAST_RESULTS
    x = np.asarray(x, np.float32)
    Wq = np.asarray(Wq, np.float32)
    Wk = np.asarray(Wk, np.float32)
    Wv = np.asarray(Wv, np.float32)
    Wp = np.asarray(Wp, np.float32)
    bp = np.asarray(bp, np.float32)

    nc = build_program()
    in_maps = [_pack_core_inputs(c, x, Wq, Wk, Wv, Wp) for c in range(NCORES)]
    kres = run_bass_kernel_spmd(nc, in_maps, list(range(NCORES)),
                                trace=_trace)
    LAST_RESULTS = kres
    res = kres.results

    out = np.empty((B, T, C), np.float32)
    for b in range(B):
        acc = np.zeros((T, C), np.float64)
        for g in range(4):
            acc += res[4 * b + g]["out"]
        out[b] = (acc + bp.astype(np.float64)).astype(np.float32)
    return out


# revision 8
# speedup vs baseline: 1.3838x; 1.0055x over previous
"""Multi-head causal attention (B=2, T=2048, C=1024, H=16, HD=64) on 8 TRN2 cores.

Sharding: core i -> batch b = i // 4, head-group g = i % 4 (heads 4g..4g+3).
Each core computes q/k/v projections for its 4 heads, causal softmax
attention, and a PARTIAL output projection against its slice of Wp.
Host sums the 4 partial projections per batch and adds the bias.

Device layout (per core):
  xT   [C, T]      x[b] transposed on host, fp16, packed (tb, ch)-major.
  qT/kT stored per head-PAIR as [128, T] fp16 (two heads on partitions).
  scores computed transposed: St[s, t] = k @ qT  (lhsT = kT, rhs = qT).
  P = exp(St * scale); causal via block skipping + windowed strided
  exp + a [128,128] lower-tri mask multiply on the diagonal 128-col strip.
  AV matmul: lhsT = V_ext [s, 65] (V plus a ones column), rhs = P[s, t]
  -> outT_ext [65, t] accumulated over s-chunks in PSUM; row 64 is the
  softmax denominator. Normalize with reciprocal + partition_broadcast.
  Projection: lhsT = outT pair chunks [128, t-tile], rhs = Wp rows; the
  fp16 partial outputs are summed on the host in float64.

Scheduling: the attention stream is a single flat software pipeline over
all (t-block, pair, s-tile) iterations: scores/exp run 2 iterations ahead
of the AV matmuls (across pair and t-block boundaries), so the ACT engine
never drains. qkv/projection matmuls are emitted as paced filler between
attention iterations to soak up PE slack. ~64 warm-up matmuls on a
memset scratch tile run during the initial DMA so HAM un-throttles the
PE clock before real work starts.
"""

import numpy as np
from collections import deque
from contextlib import ExitStack

import concourse.bass as bass
from concourse import bacc
import concourse.mybir as mybir
import concourse.tile as tile
from concourse.bass_utils import run_bass_kernel_spmd

B, T, C, H, HD = 2, 2048, 1024, 16, 64
NCORES = 8
NH = 4               # heads per core
NPAIR = 2            # head pairs per core
NCH = C // 128       # 8 contraction chunks of 128
TBW = 512            # t-block width for scores/AV
NTB = T // TBW       # 4
NST = T // 128       # 16 s-tiles
SCALE = float(HD) ** -0.5

f32 = mybir.dt.float32
f16 = mybir.dt.float16
f8 = mybir.dt.float8e4
DR = mybir.MatmulPerfMode.DoubleRow
AF = mybir.ActivationFunctionType

# exec results of the last run (exec_time_ns etc.), for test harnesses
LAST_RESULTS = None


def build_program() -> bass.Bass:
    nc = bacc.Bacc("TRN2", target_bir_lowering=False, debug=False)

    xT_d = nc.dram_tensor("xT", [128, NTB * NCH * TBW], f16,
                          kind="ExternalInput")
    wqk_d = nc.dram_tensor("wqk", [128, NPAIR * 2 * NCH * 128], f16,
                           kind="ExternalInput")
    wv_d = nc.dram_tensor("wv", [128, NCH * NH * HD], f16,
                          kind="ExternalInput")
    wp_d = nc.dram_tensor("wp", [128, NPAIR * C], f16, kind="ExternalInput")
    tri_d = nc.dram_tensor("tri", [128, 128], f16, kind="ExternalInput")
    out_d = nc.dram_tensor("out", [T, C], f16, kind="ExternalOutput")

    with tile.TileContext(nc) as tc:
        with ExitStack() as ctx:
            persist = ctx.enter_context(tc.tile_pool(name="persist", bufs=1))
            pt_pool = ctx.enter_context(tc.tile_pool(name="pt", bufs=4))
            rec_pool = ctx.enter_context(tc.tile_pool(name="rec", bufs=2))
            bc_pool = ctx.enter_context(tc.tile_pool(name="bc", bufs=2))
            pjs_pool = ctx.enter_context(tc.tile_pool(name="pjs", bufs=3))
            # PSUM budget (8 banks): warm 1 + ps512 2 + ps_sc 4 + ps_av 2
            # (warm shares with ps512's first slot after startup)
            ps512 = ctx.enter_context(
                tc.tile_pool(name="ps512", bufs=2, space="PSUM"))
            ps_sc = ctx.enter_context(
                tc.tile_pool(name="ps_sc", bufs=2, space="PSUM"))
            ps_av = ctx.enter_context(
                tc.tile_pool(name="ps_av", bufs=2, space="PSUM"))

            # ---- persistent SBUF tensors; DMA order = criticality ----
            wqk_sb = persist.tile([128, NPAIR * 2 * NCH * 128], f16, tag="wqk")
            nc.sync.dma_start(wqk_sb[:], wqk_d[:])
            xT_sb = [persist.tile([128, NCH * TBW], f16, tag=f"xT{tb}",
                                  name=f"xT{tb}") for tb in range(NTB)]
            # tb0 split in two so the first q/k matmuls start early
            for h in range(2):
                nc.sync.dma_start(
                    xT_sb[0][:, h * 4 * TBW:(h + 1) * 4 * TBW],
                    xT_d[:, h * 4 * TBW:(h + 1) * 4 * TBW])
            tri_sb = persist.tile([128, 128], f16, tag="tri")
            nc.sync.dma_start(tri_sb[:], tri_d[:])
            wv_sb = persist.tile([128, NCH * NH * HD], f16, tag="wv")
            nc.sync.dma_start(wv_sb[:], wv_d[:])
            nc.sync.dma_start(xT_sb[1][:], xT_d[:, NCH * TBW:2 * NCH * TBW])
            wp_sb = persist.tile([128, NPAIR * C], f16, tag="wp")
            nc.sync.dma_start(wp_sb[:], wp_d[:])
            for tb in range(2, NTB):
                nc.sync.dma_start(
                    xT_sb[tb][:],
                    xT_d[:, tb * NCH * TBW:(tb + 1) * NCH * TBW])

            qkT = [[persist.tile([128, T], f16, tag=f"qk{p}{t_i}",
                                 name=f"qk{p}{t_i}")
                    for t_i in range(2)] for p in range(NPAIR)]
            v_sb = [persist.tile([128, NH * 65], f16, tag=f"v{st}",
                                 name=f"v{st}")
                    for st in range(NST)]
            # ones columns for the softmax denominator: set once
            for st in range(NST):
                vt3 = v_sb[st][:].rearrange("p (h e) -> p h e", e=65)
                nc.vector.memset(vt3[:, :, 64:65], 1.0)
            oT = [persist.tile([128, T], f16, tag=f"o{p}", name=f"o{p}")
                  for p in range(NPAIR)]

            wqk3 = wqk_sb[:].rearrange("p (a b c m) -> p a b c m",
                                       a=NPAIR, b=2, c=NCH)
            wv3 = wv_sb[:].rearrange("p (c n) -> p c n", c=NCH)
            wp3 = wp_sb[:].rearrange("p (a n) -> p a n", a=NPAIR)
            xT3 = [x_[:].rearrange("p (c w) -> p c w", c=NCH) for x_ in xT_sb]

            # ---- PE warm-up: scratch tile needs no DMA, so the PE starts
            # (and HAM un-throttles) during the input DMA window ----
            wsrc = persist.tile([128, 128], f16, tag="wsrc")
            nc.vector.memset(wsrc[:], 0.25)
            warm = ps512.tile([128, TBW], f32, tag="mm", name="warm")
            for _ in range(64):
                nc.tensor.matmul(warm[:, 0:128], wsrc[:], wsrc[:],
                                 start=True, stop=True)

            # ---- filler machinery ----
            state = {"pe": 0.0, "act": 0.0}
            filler = deque()      # (tag_tb, pe_cost_ns, emit_fn)
            qk_done = {}          # (tb, p) -> bool

            def emit_qk(tb, p, t_i):
                def fn():
                    ps = ps512.tile([128, TBW], f32, tag="mm", name="ps")
                    for ch in range(NCH):
                        nc.tensor.matmul(
                            ps[:], wqk3[:, p, t_i, ch, :],
                            xT3[tb][:, ch, :],
                            start=(ch == 0), stop=(ch == NCH - 1))
                    nc.vector.tensor_copy(
                        qkT[p][t_i][:, tb * TBW:(tb + 1) * TBW], ps[:])
                    if t_i == 1:
                        qk_done[(tb, p)] = True
                return fn

            def emit_v(st):
                def fn():
                    tb = st // 4
                    j = st % 4
                    ps = ps512.tile([128, TBW], f32, tag="mm", name="ps")
                    for ch in range(NCH):
                        nc.tensor.matmul(
                            ps[:, 0:NH * HD],
                            xT3[tb][:, ch, j * 128:(j + 1) * 128],
                            wv3[:, ch, :],
                            start=(ch == 0), stop=(ch == NCH - 1))
                    vt3 = v_sb[st][:].rearrange("p (h e) -> p h e", e=65)
                    nc.vector.tensor_copy(
                        vt3[:, :, 0:64],
                        ps[:, 0:NH * HD].rearrange("p (h e) -> p h e", e=64))
                return fn

            def emit_proj(tt):
                def fn():
                    pjs = pjs_pool.tile([128, 2 * TBW], f16, tag="pjs")
                    for cb in range(2):
                        pj = ps512.tile([128, TBW], f32, tag="mm", name="pj")
                        for p in range(NPAIR):
                            nc.tensor.matmul(
                                pj[:],
                                oT[p][:, tt * 128:(tt + 1) * 128],
                                wp3[:, p, cb * TBW:(cb + 1) * TBW],
                                start=(p == 0), stop=(p == NPAIR - 1))
                        if cb == 0:
                            nc.vector.tensor_copy(
                                pjs[:, cb * TBW:(cb + 1) * TBW], pj[:])
                        else:
                            nc.scalar.copy(
                                pjs[:, cb * TBW:(cb + 1) * TBW], pj[:])
                    nc.sync.dma_start(out_d[tt * 128:(tt + 1) * 128, :],
                                      pjs[:])
                return fn

            def queue_qkv(tb):
                for t_i in range(2):
                    filler.append((tb, 1730, emit_qk(tb, 0, t_i)))
                for st in range(4 * tb, 4 * tb + 4):
                    filler.append((tb, 1050, emit_v(st)))
                for t_i in range(2):
                    filler.append((tb, 1730, emit_qk(tb, 1, t_i)))
                qk_done[(tb, 0)] = False
                qk_done[(tb, 1)] = False

            def pop1():
                tag, cost, fn = filler.popleft()
                fn()
                state["pe"] += cost

            def pop_budget():
                while filler and \
                        state["pe"] + filler[0][1] < state["act"] - 100:
                    pop1()

            # ---- flat attention pipeline over all (tb, p, st) ----
            rps = [slice(0, 64), slice(64, 128)]
            items = [(tb, p, st)
                     for tb in range(NTB)
                     for p in range(NPAIR)
                     for st in range(4 * (tb + 1))]
            pts = {}        # (tb,p,st) -> (pt_tile, lo)
            opss_by = {}    # (tb,p) -> [av tiles]
            queued = set()
            queue_qkv(0)
            queued.add(0)

            def sc_exp(it):
                tb, p, st = it
                if (tb, p) not in qk_done or not qk_done[(tb, p)]:
                    while not qk_done.get((tb, p), False):
                        pop1()
                if tb + 1 < NTB and tb + 1 not in queued:
                    queue_qkv(tb + 1)
                    queued.add(tb + 1)
                qT, kT = qkT[p][0], qkT[p][1]
                jrel = st - 4 * tb
                lo = max(jrel, 0) * 128
                scp = ps_sc.tile([128, 2 * TBW], f32, tag="sc", name="scp")
                for sub in range(2):
                    nc.tensor.matmul(
                        scp[:, sub * TBW + lo:(sub + 1) * TBW],
                        kT[rps[sub], st * 128:(st + 1) * 128],
                        qT[rps[sub], tb * TBW + lo:(tb + 1) * TBW],
                        start=True, stop=True,
                        tile_position=(sub * 64, 0))
                pt = pt_pool.tile([128, 2 * TBW], f16, tag="pt")
                pt3 = pt[:].rearrange("p (s w) -> p s w", s=2)
                scp3 = scp[:].rearrange("p (s w) -> p s w", s=2)
                nc.scalar.activation(pt3[:, :, lo:TBW], scp3[:, :, lo:TBW],
                                     AF.Exp, scale=SCALE)
                pts[it] = (pt, lo)
                state["pe"] += 2 * (TBW - lo) / 2.4 + 40
                state["act"] += (2 * (TBW - lo) + 579) / 1.2

            def av(it):
                tb, p, st = it
                nst = 4 * (tb + 1)
                pt, lo = pts.pop(it)
                if st == 0:
                    opss_by[(tb, p)] = [
                        ps_av.tile([128, TBW], f32, tag="av",
                                   name=f"av{tb}{p}{s}") for s in range(2)]
                opss = opss_by[(tb, p)]
                jrel = st - 4 * tb
                if jrel >= 0:
                    for sub in range(2):
                        nc.vector.tensor_mul(
                            pt[:, sub * TBW + lo:sub * TBW + lo + 128],
                            pt[:, sub * TBW + lo:sub * TBW + lo + 128],
                            tri_sb[:])
                vt3 = v_sb[st][:].rearrange("p (h e) -> p h e", e=65)
                for sub in range(2):
                    nc.tensor.matmul(
                        opss[sub][0:65, lo:TBW],
                        vt3[:, 2 * p + sub, :],
                        pt[:, sub * TBW + lo:(sub + 1) * TBW],
                        start=(st == 0), stop=(st == nst - 1))
                state["pe"] += 2 * (TBW - lo) / 2.4 + 90

            def normalize(tb, p):
                tsl = slice(tb * TBW, (tb + 1) * TBW)
                opss = opss_by.pop((tb, p))
                for sub in range(2):
                    ops = opss[sub]
                    rs = rec_pool.tile([1, TBW], f32, tag="rs")
                    nc.vector.tensor_copy(rs[:], ops[64:65, :])
                    rec = rec_pool.tile([1, TBW], f32, tag="rec")
                    nc.vector.reciprocal_approx_fast(rec[:], rs[:])
                    bcb = bc_pool.tile([64, TBW], f32, tag="bc")
                    nc.gpsimd.partition_broadcast(bcb[:], rec[:])
                    nc.vector.tensor_mul(oT[p][rps[sub], tsl],
                                         ops[0:64, :], bcb[:])

            LEAD = 2
            sc_exp(items[0])
            sc_exp(items[1])
            for i, it in enumerate(items):
                if i + LEAD < len(items):
                    sc_exp(items[i + LEAD])
                if filler and filler[0][0] <= it[0] + 1:
                    pop1()
                av(it)
                tb, p, st = it
                if st == 4 * (tb + 1) - 1:
                    normalize(tb, p)
                    if p == 1:
                        for tt in range(4 * tb, 4 * tb + 4):
                            filler.append((tb + 1, 900, emit_proj(tt)))
                pop_budget()

            while filler:
                pop1()

    nc.compile()
    return nc


def _pack_core_inputs(core, x, Wq, Wk, Wv, Wp):
    b, g = core // 4, core % 4
    hs = [4 * g + i for i in range(NH)]

    xT = np.ascontiguousarray(x[b].T)  # [C, T]
    xTp = np.empty((128, NTB, NCH, TBW), np.float32)
    for tb in range(NTB):
        for ch in range(NCH):
            xTp[:, tb, ch, :] = xT[ch * 128:(ch + 1) * 128,
                                   tb * TBW:(tb + 1) * TBW]

    wqk = np.empty((128, NPAIR, 2, NCH, 128), np.float32)
    for p in range(NPAIR):
        for t_i, W in enumerate((Wq, Wk)):
            pair = np.concatenate([W[hs[2 * p]], W[hs[2 * p + 1]]], axis=1)
            for ch in range(NCH):
                wqk[:, p, t_i, ch, :] = pair[ch * 128:(ch + 1) * 128, :]

    wv4 = np.concatenate([Wv[h] for h in hs], axis=1)  # [C, 256]
    wv = np.empty((128, NCH, NH * HD), np.float32)
    for ch in range(NCH):
        wv[:, ch, :] = wv4[ch * 128:(ch + 1) * 128]

    wp = np.empty((128, NPAIR, C), np.float32)
    for p in range(NPAIR):
        rows = np.r_[hs[2 * p] * HD:(hs[2 * p] + 1) * HD,
                     hs[2 * p + 1] * HD:(hs[2 * p + 1] + 1) * HD]
        wp[:, p, :] = Wp[rows, :]

    c_idx = np.arange(128)[None, :]
    s_idx = np.arange(128)[:, None]
    tri = (c_idx >= s_idx)  # [128, 128] lower-tri inclusive, t >= s

    return {
        "xT": np.ascontiguousarray(xTp.reshape(128, -1)).astype(np.float16),
        "wqk": np.ascontiguousarray(wqk.reshape(128, -1)).astype(np.float16),
        "wv": np.ascontiguousarray(wv.reshape(128, -1)).astype(np.float16),
        "wp": np.ascontiguousarray(wp.reshape(128, -1)).astype(np.float16),
        "tri": tri.astype(np.float16),
    }


def kernel(x, Wq, Wk, Wv, Wp, bp, _trace=False):
    global LAST_RESULTS
    x = np.asarray(x, np.float32)
    Wq = np.asarray(Wq, np.float32)
    Wk = np.asarray(Wk, np.float32)
    Wv = np.asarray(Wv, np.float32)
    Wp = np.asarray(Wp, np.float32)
    bp = np.asarray(bp, np.float32)

    nc = build_program()
    in_maps = [_pack_core_inputs(c, x, Wq, Wk, Wv, Wp) for c in range(NCORES)]
    kres = run_bass_kernel_spmd(nc, in_maps, list(range(NCORES)),
                                trace=_trace)
    LAST_RESULTS = kres
    res = kres.results

    out = np.empty((B, T, C), np.float32)
    for b in range(B):
        acc = np.zeros((T, C), np.float64)
        for g in range(4):
            acc += res[4 * b + g]["out"].astype(np.float64)
        out[b] = (acc + bp.astype(np.float64)).astype(np.float32)
    return out
